# revision 5
# baseline (speedup 1.0000x reference)
"""GCN (2x GraphConv + mean-pool + sigmoid) as a Trainium2 Bass kernel.

Strategy (8-way SPMD, graph/data parallel over dst nodes):
  - Edges are partitioned by dst-owner core and grouped into 128-dst-node
    tiles. For each tile, gathered src feature rows (bf16) are aggregated
    per dst node with a one-hot matmul on the PE:
        aggT[f, d] = sum_e feat[src_e, f] * (dstloc_e == d)
    Linearity lets us aggregate raw features first, then transform:
        x1 = relu(aggT.T @ W1)
  - Layer 2 + mean-pool fold into a per-node COUNT matrix:
        pooled[g] = sum_d in g x2[d] = sum_n COUNT[g, n] * (x1 @ W2)[n]
    where COUNT[g, n] = #edges from node n into graph g. The device
    accumulates poolT = sum_tiles x1_tile.T @ COUNT_tile.T; the tiny
    final (@W2 @Wfc, /cnt, sigmoid) runs on host in f64.
  - Each core outputs its partial poolT [64, 128]; host sums the 8.
"""

import numpy as np
import ml_dtypes

P = 128
N_FEATS = 128
DIM = 64
N_GRAPHS = 128
N_CORES = 8
PAD_DST = 200.0  # dst-local id that never matches iota 0..127; exact in bf16


def _ceil_div(a, b):
    return -(-a // b)


def _preprocess(features, src, dst, graph_ids, W1, n_cores=N_CORES):
    n_nodes, F = features.shape
    G = N_GRAPHS
    nodes_per_core = _ceil_div(_ceil_div(n_nodes, n_cores), P) * P
    nodes_pad = nodes_per_core * n_cores
    tiles = nodes_per_core // P

    order = np.argsort(dst)
    srcs = src[order].astype(np.int64)
    dsts = dst[order].astype(np.int64)
    bounds = np.searchsorted(dsts, np.arange(0, nodes_pad + P, P)).astype(np.int64)
    cnts = np.diff(bounds).reshape(n_cores, tiles)
    C = _ceil_div(cnts, P)  # chunks of 128 edges per (core, tile)
    # SPMD needs identical chunk counts per loop iteration on every core:
    # order each core's tiles by descending C, pad to the rank-wise max.
    perm = np.argsort(-C, axis=1, kind="stable")
    Csort = np.take_along_axis(C, perm, axis=1)
    C_rank = np.maximum(Csort.max(axis=0), 1).astype(np.int64)
    offs = np.concatenate([[0], np.cumsum(C_rank * P)])
    total_slots = int(offs[-1])

    bf16 = ml_dtypes.bfloat16
    cntT = np.bincount(
        src.astype(np.int64) * G + graph_ids[dst].astype(np.int64),
        minlength=n_nodes * G,
    ).reshape(n_nodes, G)
    assert cntT.max() < 256  # exact in bf16
    cntT_pad = np.zeros((nodes_pad, G), np.float32)
    cntT_pad[:n_nodes] = cntT
    feat_bf = np.ascontiguousarray(features.astype(bf16))
    w1_bf = np.ascontiguousarray(W1.astype(bf16))

    in_maps = []
    for c in range(n_cores):
        gidx = np.zeros(total_slots, np.int32)
        gdst = np.full(total_slots, PAD_DST, np.float32)
        for r in range(tiles):
            t = int(perm[c, r])
            gt = c * tiles + t
            lo, hi = int(bounds[gt]), int(bounds[gt + 1])
            cnt = hi - lo
            Cr = int(C_rank[r])
            si = np.zeros(Cr * P, np.int64)
            sd = np.full(Cr * P, PAD_DST, np.float64)
            si[:cnt] = srcs[lo:hi]
            sd[:cnt] = dsts[lo:hi] - (c * nodes_per_core + t * P)
            o = int(offs[r])
            # slot (partition p, chunk k) holds edge k*128+p; DRAM layout
            # is [128, Cr] row-major per tile.
            gidx[o : o + Cr * P] = si.reshape(Cr, P).T.ravel()
            gdst[o : o + Cr * P] = sd.reshape(Cr, P).T.ravel()
        rows = (
            c * nodes_per_core
            + (perm[c][:, None] * P + np.arange(P)[None, :]).ravel()
        )
        in_maps.append(
            {
                "featbf": feat_bf,
                "gidx": gidx,
                "gdst": gdst.astype(bf16),
                "countT": np.ascontiguousarray(cntT_pad[rows].astype(bf16)),
                "w1": w1_bf,
            }
        )
    meta = dict(
        tiles=tiles,
        C_rank=[int(x) for x in C_rank],
        offs=[int(x) for x in offs],
        total_slots=total_slots,
        n_table=n_nodes,
        F=F,
        D=W1.shape[1],
        G=G,
    )
    return in_maps, meta


def _legalize_single_wait(nc, mybir):
    # The walrus codegen in this environment rejects instructions carrying
    # more than one semaphore wait ("Too many sync wait commands").
    # TileContext emits multi-wait instructions (notably the exit drain);
    # hoist all but the last wait onto single-wait NoOps on the same engine.
    ctr = 0
    for func in nc.m.functions:
        for blk in func.blocks:
            insts = blk.instructions
            i = 0
            while i < len(insts):
                inst = insts[i]
                si = inst.sync_info
                if si is not None and len(si.on_wait) > 1:
                    waits = list(si.on_wait)
                    for w in waits[:-1]:
                        nop = mybir.InstNoOp(
                            name=f"{inst.name}-sw{ctr}",
                            engine=inst.engine,
                            ins=[],
                            outs=[],
                            sync_info=mybir.SyncInfo(on_wait=[w], on_update=[]),
                        )
                        nop.bass_nofuse = True
                        insts.insert(i, nop)
                        ctr += 1
                        i += 1
                    inst.sync_info = mybir.SyncInfo(
                        on_wait=[waits[-1]], on_update=list(si.on_update)
                    )
                i += 1
    return ctr


def _build(meta, n_cores=N_CORES):
    from contextlib import ExitStack

    import concourse.tile as tile
    from concourse import bass, mybir

    F, D, G = meta["F"], meta["D"], meta["G"]
    tiles = meta["tiles"]
    C_rank = meta["C_rank"]
    offs = meta["offs"]
    bf = mybir.dt.bfloat16
    f32 = mybir.dt.float32
    AFT = mybir.ActivationFunctionType

    nc = bass.Bass(
        "TRN2", target_bir_lowering=False, debug=False, num_devices=n_cores
    )
    feat = nc.dram_tensor("featbf", [meta["n_table"], F], bf, kind="ExternalInput")
    gidx = nc.dram_tensor("gidx", [meta["total_slots"]], mybir.dt.int32,
                          kind="ExternalInput")
    gdst = nc.dram_tensor("gdst", [meta["total_slots"]], bf, kind="ExternalInput")
    countT = nc.dram_tensor("countT", [tiles * P, G], bf, kind="ExternalInput")
    w1 = nc.dram_tensor("w1", [F, D], bf, kind="ExternalInput")
    pool_out = nc.dram_tensor("pool_out", [D, G], f32, kind="ExternalOutput")

    with ExitStack() as ctx:
        tc = ctx.enter_context(tile.TileContext(nc))
        const = ctx.enter_context(tc.tile_pool(name="const", bufs=1))
        sb_g = ctx.enter_context(tc.tile_pool(name="sbg", bufs=2))
        sb_o = ctx.enter_context(tc.tile_pool(name="sbo", bufs=2))
        sb_s = ctx.enter_context(tc.tile_pool(name="sbs", bufs=3))
        ps_a = ctx.enter_context(tc.tile_pool(name="psa", bufs=2, space="PSUM"))
        ps_x = ctx.enter_context(tc.tile_pool(name="psx", bufs=2, space="PSUM"))
        ps_p = ctx.enter_context(tc.tile_pool(name="psp", bufs=1, space="PSUM"))

        w1_s = const.tile([F, D], dtype=bf)
        nc.sync.dma_start(out=w1_s[:], in_=w1[:, :])
        iota_i = const.tile([P, P], dtype=mybir.dt.int32)
        nc.gpsimd.iota(iota_i[:], [[1, P]], channel_multiplier=0)
        iota_b = const.tile([P, P], dtype=bf)
        nc.vector.tensor_copy(out=iota_b[:], in_=iota_i[:])
        poolacc = ps_p.tile([D, G], dtype=f32, space="PSUM")

        for r in range(tiles):
            C = C_rank[r]
            off = offs[r]
            idx_t = sb_s.tile([P, C], dtype=mybir.dt.int32)
            dst_t = sb_s.tile([P, C], dtype=bf)
            nc.sync.dma_start(out=idx_t[:], in_=bass.AP(gidx, off, [[C, P], [1, C]]))
            nc.sync.dma_start(out=dst_t[:], in_=bass.AP(gdst, off, [[C, P], [1, C]]))
            gf = sb_g.tile([P, C * F], dtype=bf)
            for k in range(C):
                nc.gpsimd.indirect_dma_start(
                    out=gf[:, k * F : (k + 1) * F],
                    out_offset=None,
                    in_=feat[:, :],
                    in_offset=bass.IndirectOffsetOnAxis(
                        ap=idx_t[:, k : k + 1], axis=0
                    ),
                )
            oh = sb_o.tile([P, C * P], dtype=bf)
            for k in range(C):
                nc.vector.tensor_tensor(
                    out=oh[:, k * P : (k + 1) * P],
                    in0=dst_t[:, k : k + 1].to_broadcast([P, P]),
                    in1=iota_b[:],
                    op=mybir.AluOpType.is_equal,
                )
            aggT = ps_a.tile([F, P], dtype=f32, space="PSUM")
            for k in range(C):
                nc.tensor.matmul(
                    out=aggT[:],
                    lhsT=gf[:, k * F : (k + 1) * F],
                    rhs=oh[:, k * P : (k + 1) * P],
                    start=(k == 0),
                    stop=(k == C - 1),
                )
            aggT_s = sb_s.tile([F, P], dtype=bf)
            nc.scalar.activation(out=aggT_s[:], in_=aggT[:], func=AFT.Copy)
            x1p = ps_x.tile([P, D], dtype=f32, space="PSUM")
            nc.tensor.matmul(out=x1p[:], lhsT=aggT_s[:], rhs=w1_s[:],
                             start=True, stop=True)
            x1_s = sb_s.tile([P, D], dtype=bf)
            nc.scalar.activation(out=x1_s[:], in_=x1p[:], func=AFT.Relu)
            cnt_t = sb_s.tile([P, G], dtype=bf)
            nc.scalar.dma_start(out=cnt_t[:], in_=countT[r * P : (r + 1) * P, :])
            nc.tensor.matmul(
                out=poolacc[:],
                lhsT=x1_s[:],
                rhs=cnt_t[:],
                start=(r == 0),
                stop=(r == tiles - 1),
            )
        out_s = const.tile([D, G], dtype=f32)
        nc.vector.tensor_copy(out=out_s[:], in_=poolacc[:])
        nc.sync.dma_start(out=pool_out[:, :], in_=out_s[:])
    _legalize_single_wait(nc, mybir)
    return nc


def _postprocess(partials, graph_ids, W2, Wfc):
    poolT = np.zeros_like(partials[0], dtype=np.float64)
    for p in partials:
        poolT += np.asarray(p, np.float64)
    P1 = poolT.T  # [G, D]
    cnt = np.bincount(graph_ids, minlength=N_GRAPHS).astype(np.float64)
    z = (P1 @ W2.astype(np.float64) @ Wfc.astype(np.float64)) / np.maximum(
        cnt, 1.0
    )[:, None]
    return (1.0 / (1.0 + np.exp(-z))).astype(np.float32)


def kernel(**inputs):
    features = np.asarray(inputs["features"], np.float32)
    src = np.asarray(inputs["src"])
    dst = np.asarray(inputs["dst"])
    graph_ids = np.asarray(inputs["graph_ids"])
    W1 = np.asarray(inputs["W1"], np.float32)
    W2 = np.asarray(inputs["W2"], np.float32)
    Wfc = np.asarray(inputs["Wfc"], np.float32)

    in_maps, meta = _preprocess(features, src, dst, graph_ids, W1)
    nc = _build(meta)
    from concourse.bass_utils import run_bass_kernel_spmd

    res = run_bass_kernel_spmd(nc, in_maps, list(range(N_CORES)))
    partials = [r["pool_out"] for r in res.results]
    return _postprocess(partials, graph_ids, W2, Wfc)
